# revision 2
# baseline (speedup 1.0000x reference)
"""Low_Rank_linear Trainium2 kernel.

Math (reference):
    hidden = (x[..., col_idx] * wnorm) @ B.T            # [tok, 512]
    y[..., row_idx]      = hidden @ A.T + x[..., col_comp_idx] @ sparse1.T
    y[..., row_comp_idx] = x @ sparse2.T

Reformulation used here (all index handling folded into host-built weights):
    u = x @ W1.T        W1 = [Bs; G; sparse2]  (1024 x 4096)
        Bs[:, col_idx]        = B * wnorm      (rank rows scattered to full width)
        G[i, col_comp_idx[i]] = 1              (one-hot gather of comp columns)
    y = u @ W2.T        W2 (4096 x 1024), rows interleaved on host:
        W2[row_idx[j]]      = [A[j] | sparse1[j] | 0]
        W2[row_comp_idx[i]] = [0    | 0          | e_i]
    so y comes out of the second matmul already in natural column order.

Sharding: data-parallel over the 8192 tokens -> 1024 tokens per core, weights
replicated. No collectives. Matmuls run in bf16 with fp32 PSUM accumulation.
"""

import numpy as np
import ml_dtypes

import concourse.bacc as bacc
import concourse.tile as tile
import concourse.mybir as mybir
from concourse.bass_utils import run_bass_kernel_spmd

N_CORES = 8
TOK = 8192            # 4 * 2048 tokens total
TPC = TOK // N_CORES  # 1024 tokens per core
N = 4096              # model width (in == out)
RANK = 512
NCOMP = 256           # complement set size (both col and row)
KU = RANK + NCOMP + NCOMP  # 1024 = width of intermediate u
BLK = 512             # token block (matmul moving N)
TT = 128              # token tile (partition dim)

_BF16 = mybir.dt.bfloat16
_F32 = mybir.dt.float32


def _build_nc():
    nc = bacc.Bacc(None)
    x_d = nc.dram_tensor("x", [TPC, N], _F32, kind="ExternalInput")
    w1_d = nc.dram_tensor("w1t", [N, KU], _BF16, kind="ExternalInput")
    w2_d = nc.dram_tensor("w2t", [KU, N], _BF16, kind="ExternalInput")
    y_d = nc.dram_tensor("y", [TPC, N], _F32, kind="ExternalOutput")

    n_blk = TPC // BLK          # 2 token blocks per core
    tpb = BLK // TT             # 4 token tiles per block
    k1 = N // 128               # 32 k-tiles for matmul A
    m1 = KU // 128              # 8 u-feature tiles
    k2 = KU // 128              # 8 k-tiles for matmul B
    n2 = N // BLK               # 8 output-feature chunks

    with tile.TileContext(nc) as tc:
        with (
            tc.tile_pool(name="w1", bufs=1) as w1_pool,
            tc.tile_pool(name="w2", bufs=2) as w2_pool,
            tc.tile_pool(name="xb", bufs=2) as xb_pool,
            tc.tile_pool(name="xt", bufs=2) as xt_pool,
            tc.tile_pool(name="u3", bufs=2) as u3_pool,
            tc.tile_pool(name="yo", bufs=4) as yo_pool,
            tc.tile_pool(name="psA", bufs=2, space="PSUM") as psA,
            tc.tile_pool(name="psB", bufs=2, space="PSUM") as psB,
        ):
            # resident W1.T in SBUF: [128, 32 k-tiles, 1024]
            w1_sb = w1_pool.tile([128, k1, KU], _BF16)
            nc.sync.dma_start(
                w1_sb[:], w1_d.rearrange("(kt p) m -> p kt m", p=128)
            )

            for blk in range(n_blk):
                t0 = blk * BLK
                # load + cast x to bf16 (token-major), then DMA-transpose to
                # feature-major xt [128 feat, k-tile, 512 tok]
                xt_sb = xt_pool.tile([128, k1, BLK], _BF16)
                for tt in range(tpb):
                    xb = xb_pool.tile([128, N], _BF16)
                    nc.gpsimd.dma_start(
                        xb[:], x_d[t0 + tt * TT : t0 + (tt + 1) * TT, :]
                    )
                    for kt in range(k1):
                        nc.sync.dma_start_transpose(
                            xt_sb[:, kt, tt * TT : (tt + 1) * TT],
                            xb[:, kt * 128 : (kt + 1) * 128],
                        )

                # MM-A: u.T [ufeat, tok] = W1 @ x.T ; cast to bf16
                u3_sb = u3_pool.tile([128, k2, BLK], _BF16)
                for m in range(m1):
                    ps = psA.tile([128, BLK], _F32)
                    for kt in range(k1):
                        nc.tensor.matmul(
                            ps[:],
                            w1_sb[:, kt, m * 128 : (m + 1) * 128],
                            xt_sb[:, kt, :],
                            start=(kt == 0),
                            stop=(kt == k1 - 1),
                        )
                    nc.vector.tensor_copy(out=u3_sb[:, m, :], in_=ps[:])

                # MM-B: y [tok, outfeat] = u @ W2.T, n-chunk at a time
                for n in range(n2):
                    w2_sb = w2_pool.tile([128, k2, BLK], _BF16)
                    nc.sync.dma_start(
                        w2_sb[:],
                        w2_d.rearrange("(kt p) n -> p kt n", p=128)[
                            :, :, n * BLK : (n + 1) * BLK
                        ],
                    )
                    for mt in range(tpb):
                        ps = psB.tile([128, BLK], _F32)
                        for kt in range(k2):
                            nc.tensor.matmul(
                                ps[:],
                                u3_sb[:, kt, mt * TT : (mt + 1) * TT],
                                w2_sb[:, kt, :],
                                start=(kt == 0),
                                stop=(kt == k2 - 1),
                            )
                        yo = yo_pool.tile([128, BLK], _F32)
                        nc.vector.tensor_copy(out=yo[:], in_=ps[:])
                        nc.sync.dma_start(
                            y_d[
                                t0 + mt * TT : t0 + (mt + 1) * TT,
                                n * BLK : (n + 1) * BLK,
                            ],
                            yo[:],
                        )
    nc.finalize()
    return nc


_NC_CACHE = {}


def get_nc():
    if "nc" not in _NC_CACHE:
        _NC_CACHE["nc"] = _build_nc()
    return _NC_CACHE["nc"]


def _prep_weights(A, B, sparse_weights1, sparse_weights2, weights_norms_rowwise,
                  col_idx, col_comp_idx, row_idx, row_comp_idx):
    bf16 = ml_dtypes.bfloat16
    # W1 = [Bs; G; sparse2]  (1024, 4096)
    w1 = np.zeros((KU, N), dtype=np.float32)
    w1[:RANK, col_idx] = B * weights_norms_rowwise[None, :]
    w1[RANK + np.arange(NCOMP), col_comp_idx] = 1.0
    w1[RANK + NCOMP :, :] = sparse_weights2
    # W2 (4096, 1024) with interleaved rows; build transposed directly
    w2t = np.zeros((KU, N), dtype=np.float32)
    w2t[:RANK, row_idx] = A.T
    w2t[RANK : RANK + NCOMP, row_idx] = sparse_weights1.T
    w2t[RANK + NCOMP + np.arange(NCOMP), row_comp_idx] = 1.0
    w1t = np.ascontiguousarray(w1.T).astype(bf16)       # [4096, 1024]
    w2t = np.ascontiguousarray(w2t).astype(bf16)        # [1024, 4096]
    return w1t, w2t


def kernel(x, A, B, sparse_weights1, sparse_weights2, weights_norms_rowwise,
           col_idx, col_comp_idx, row_idx, row_comp_idx):
    x = np.asarray(x, dtype=np.float32)
    w1t, w2t = _prep_weights(
        np.asarray(A, np.float32), np.asarray(B, np.float32),
        np.asarray(sparse_weights1, np.float32),
        np.asarray(sparse_weights2, np.float32),
        np.asarray(weights_norms_rowwise, np.float32),
        np.asarray(col_idx), np.asarray(col_comp_idx),
        np.asarray(row_idx), np.asarray(row_comp_idx),
    )
    nc = get_nc()
    xs = np.ascontiguousarray(x.reshape(TOK, N))
    in_maps = [
        {"x": xs[c * TPC : (c + 1) * TPC], "w1t": w1t, "w2t": w2t}
        for c in range(N_CORES)
    ]
    res = run_bass_kernel_spmd(nc, in_maps, core_ids=list(range(N_CORES)))
    globals()["_LAST_RESULTS"] = res
    y = np.concatenate([res.results[c]["y"] for c in range(N_CORES)], axis=0)
    return np.ascontiguousarray(y.reshape(x.shape).astype(np.float32))


# revision 7
# speedup vs baseline: 1.5649x; 1.5649x over previous
"""Low_Rank_linear Trainium2 kernel, v4.

Same math as v3 but the one-hot G matmul is replaced by a gpsimd ap_gather
of the 256 comp columns from the fp32 x tile (then cast + xbar transpose
straight into the u3 stationary buffer). MM-A shrinks from 1024 to 768
output features.

u layout (order unchanged vs v3, so W2 host prep is identical):
    u = [hidden(512) | x_comp(256) | y_comp(256)]
    MM-A computes hidden (m=0..3 -> u3 slots 0..3) and y_comp
    (m=4..5 of W1' = sparse2 -> u3 slots 6..7); x_comp arrives via gather
    into slots 4..5.
"""

import numpy as np
import ml_dtypes

import concourse.bacc as bacc
import concourse.tile as tile
import concourse.mybir as mybir
from concourse.bass_utils import run_bass_kernel_spmd

N_CORES = 8
TOK = 8192
TPC = TOK // N_CORES
N = 4096
RANK = 512
NCOMP = 256
KU = RANK + NCOMP + NCOMP   # 1024: u width (for MM-B)
KW1 = RANK + NCOMP          # 768: MM-A output features (Bs + sparse2)
BLK = 512
TT = 128

_BF16 = mybir.dt.bfloat16
_F32 = mybir.dt.float32
_I16 = mybir.dt.int16


def _build_nc():
    nc = bacc.Bacc(None)
    x_d = nc.dram_tensor("x", [TPC, N], _F32, kind="ExternalInput")
    w1_d = nc.dram_tensor("w1t", [N, KW1], _BF16, kind="ExternalInput")
    w2_d = nc.dram_tensor("w2t", [KU, N], _BF16, kind="ExternalInput")
    cci_d = nc.dram_tensor("cci", [128, 16], _I16, kind="ExternalInput")
    y_d = nc.dram_tensor("y", [TPC, N], _F32, kind="ExternalOutput")

    n_blk = TPC // BLK
    tpb = BLK // TT
    k1 = N // 128               # 32 k-tiles for MM-A
    m1 = KW1 // 128             # 6 u-feature tiles from MM-A
    k2 = KU // 128              # 8 k-tiles for MM-B
    n2 = N // BLK               # 8 output chunks
    # MM-A m-tile -> u3 slot (hidden -> 0..3, y_comp -> 6..7)
    m_slot = [0, 1, 2, 3, 6, 7]

    with tile.TileContext(nc) as tc:
        with (
            tc.tile_pool(name="w1", bufs=1) as w1_pool,
            tc.tile_pool(name="w2", bufs=2) as w2_pool,
            tc.tile_pool(name="cci", bufs=1) as cci_pool,
            tc.tile_pool(name="xf", bufs=2) as xf_pool,
            tc.tile_pool(name="xb", bufs=2) as xb_pool,
            tc.tile_pool(name="xc", bufs=2) as xc_pool,
            tc.tile_pool(name="xt", bufs=1) as xt_pool,
            tc.tile_pool(name="u3", bufs=2) as u3_pool,
            tc.tile_pool(name="yo", bufs=4) as yo_pool,
            tc.tile_pool(name="psA", bufs=2, space="PSUM") as psA,
            tc.tile_pool(name="psB", bufs=2, space="PSUM") as psB,
        ):
            w1_sb = w1_pool.tile([128, k1, KW1], _BF16)
            nc.scalar.dma_start(w1_sb[:], w1_d.rearrange("(kt p) m -> p kt m", p=128))
            cci_sb = cci_pool.tile([128, 16], _I16)
            nc.scalar.dma_start(cci_sb[:], cci_d[:])

            for blk in range(n_blk):
                t0 = blk * BLK
                xt_sb = xt_pool.tile([128, k1, BLK], _BF16)
                u3_sb = u3_pool.tile([128, k2, BLK], _BF16)
                for tt in range(tpb):
                    xf = xf_pool.tile([128, N], _F32)
                    nc.gpsimd.dma_start(
                        xf[:], x_d[t0 + tt * TT : t0 + (tt + 1) * TT, :]
                    )
                    xb = xb_pool.tile([128, N], _BF16)
                    nc.vector.tensor_copy(out=xb[:], in_=xf[:])
                    nc.sync.dma_start_transpose(
                        xt_sb[:, :, tt * TT : (tt + 1) * TT], xb[:]
                    )
                    # comp-column gather: [128 tok, 256] then cast+transpose
                    # into u3 slots 4..5
                    xc = xc_pool.tile([128, NCOMP], _F32, tag="xcf")
                    nc.gpsimd.ap_gather(
                        xc[:], xf[:], cci_sb[:],
                        channels=128, num_elems=N, d=1, num_idxs=NCOMP,
                    )
                    xcb = xc_pool.tile([128, NCOMP], _BF16, tag="xcb")
                    nc.vector.tensor_copy(out=xcb[:], in_=xc[:])
                    nc.sync.dma_start_transpose(
                        u3_sb[:, 4:6, tt * TT : (tt + 1) * TT], xcb[:]
                    )

                for mi, m in enumerate(range(m1)):
                    ps = psA.tile([128, BLK], _F32)
                    for kt in range(k1):
                        nc.tensor.matmul(
                            ps[:],
                            w1_sb[:, kt, m * 128 : (m + 1) * 128],
                            xt_sb[:, kt, :],
                            start=(kt == 0),
                            stop=(kt == k1 - 1),
                        )
                    nc.vector.tensor_copy(out=u3_sb[:, m_slot[mi], :], in_=ps[:])

                for n in range(n2):
                    w2_sb = w2_pool.tile([128, k2, BLK], _BF16)
                    nc.scalar.dma_start(
                        w2_sb[:],
                        w2_d.rearrange("(kt p) n -> p kt n", p=128)[
                            :, :, n * BLK : (n + 1) * BLK
                        ],
                    )
                    for mt in range(tpb):
                        ps = psB.tile([128, BLK], _F32)
                        for kt in range(k2):
                            nc.tensor.matmul(
                                ps[:],
                                u3_sb[:, kt, mt * TT : (mt + 1) * TT],
                                w2_sb[:, kt, :],
                                start=(kt == 0),
                                stop=(kt == k2 - 1),
                            )
                        yo = yo_pool.tile([128, BLK], _F32)
                        nc.vector.tensor_copy(out=yo[:], in_=ps[:])
                        nc.scalar.dma_start(
                            y_d[
                                t0 + mt * TT : t0 + (mt + 1) * TT,
                                n * BLK : (n + 1) * BLK,
                            ],
                            yo[:],
                        )
    nc.finalize()
    return nc


_NC_CACHE = {}


def get_nc():
    if "nc" not in _NC_CACHE:
        _NC_CACHE["nc"] = _build_nc()
    return _NC_CACHE["nc"]


def _prep_weights(A, B, sparse_weights1, sparse_weights2, weights_norms_rowwise,
                  col_idx, col_comp_idx, row_idx, row_comp_idx):
    bf16 = ml_dtypes.bfloat16
    # W1' = [Bs; sparse2]  (768, 4096)
    w1 = np.zeros((KW1, N), dtype=np.float32)
    w1[:RANK, col_idx] = B * weights_norms_rowwise[None, :]
    w1[RANK:, :] = sparse_weights2
    # W2.T (1024, 4096): u = [hidden | x_comp | y_comp]
    w2t = np.zeros((KU, N), dtype=np.float32)
    w2t[:RANK, row_idx] = A.T
    w2t[RANK : RANK + NCOMP, row_idx] = sparse_weights1.T
    w2t[RANK + NCOMP + np.arange(NCOMP), row_comp_idx] = 1.0
    w1t = np.ascontiguousarray(w1.T).astype(bf16)
    w2t = np.ascontiguousarray(w2t).astype(bf16)
    # ap_gather index tile: idx j = s*16+p read from cci[p, s]
    cci = np.asarray(col_comp_idx, np.int64).reshape(16, 16).T.astype(np.int16)
    cci = np.ascontiguousarray(np.broadcast_to(cci[None], (8, 16, 16)).reshape(128, 16))
    return w1t, w2t, cci


def kernel(x, A, B, sparse_weights1, sparse_weights2, weights_norms_rowwise,
           col_idx, col_comp_idx, row_idx, row_comp_idx):
    x = np.asarray(x, dtype=np.float32)
    w1t, w2t, cci = _prep_weights(
        np.asarray(A, np.float32), np.asarray(B, np.float32),
        np.asarray(sparse_weights1, np.float32),
        np.asarray(sparse_weights2, np.float32),
        np.asarray(weights_norms_rowwise, np.float32),
        np.asarray(col_idx), np.asarray(col_comp_idx),
        np.asarray(row_idx), np.asarray(row_comp_idx),
    )
    nc = get_nc()
    xs = np.ascontiguousarray(x.reshape(TOK, N))
    in_maps = [
        {"x": xs[c * TPC : (c + 1) * TPC], "w1t": w1t, "w2t": w2t, "cci": cci}
        for c in range(N_CORES)
    ]
    res = run_bass_kernel_spmd(nc, in_maps, core_ids=list(range(N_CORES)))
    globals()["_LAST_RESULTS"] = res
    y = np.concatenate([res.results[c]["y"] for c in range(N_CORES)], axis=0)
    return np.ascontiguousarray(y.reshape(x.shape).astype(np.float32))


# revision 8
# speedup vs baseline: 1.5993x; 1.0220x over previous
"""Low_Rank_linear Trainium2 kernel, v6.

Same math as v3 but the one-hot G matmul is replaced by a gpsimd ap_gather
of the 256 comp columns from the fp32 x tile (then cast + xbar transpose
straight into the u3 stationary buffer). MM-A shrinks from 1024 to 768
output features.

u layout (order unchanged vs v3, so W2 host prep is identical):
    u = [hidden(512) | x_comp(256) | y_comp(256)]
    MM-A computes hidden (m=0..3 -> u3 slots 0..3) and y_comp
    (m=4..5 of W1' = sparse2 -> u3 slots 6..7); x_comp arrives via gather
    into slots 4..5.
"""

import numpy as np
import ml_dtypes

import concourse.bacc as bacc
import concourse.tile as tile
import concourse.mybir as mybir
from concourse.bass_utils import run_bass_kernel_spmd

N_CORES = 8
TOK = 8192
TPC = TOK // N_CORES
N = 4096
RANK = 512
NCOMP = 256
KU = RANK + NCOMP + NCOMP   # 1024: u width (for MM-B)
KW1 = RANK + NCOMP          # 768: MM-A output features (Bs + sparse2)
BLK = 512
TT = 128

_BF16 = mybir.dt.bfloat16
_F32 = mybir.dt.float32
_I16 = mybir.dt.int16


def _build_nc():
    nc = bacc.Bacc(None)
    x_d = nc.dram_tensor("x", [TPC, N], _F32, kind="ExternalInput")
    w1_d = nc.dram_tensor("w1t", [N, KW1], _BF16, kind="ExternalInput")
    w2_d = nc.dram_tensor("w2t", [KU, N], _BF16, kind="ExternalInput")
    cci_d = nc.dram_tensor("cci", [128, 16], _I16, kind="ExternalInput")
    y_d = nc.dram_tensor("y", [TPC, N], _F32, kind="ExternalOutput")

    n_blk = TPC // BLK
    tpb = BLK // TT
    k1 = N // 128               # 32 k-tiles for MM-A
    m1 = KW1 // 128             # 6 u-feature tiles from MM-A
    k2 = KU // 128              # 8 k-tiles for MM-B
    n2 = N // BLK               # 8 output chunks
    # MM-A m-tile -> u3 slot (hidden -> 0..3, y_comp -> 6..7)
    m_slot = [0, 1, 2, 3, 6, 7]

    with tile.TileContext(nc) as tc:
        with (
            tc.tile_pool(name="w1", bufs=1) as w1_pool,
            tc.tile_pool(name="w2", bufs=2) as w2_pool,
            tc.tile_pool(name="cci", bufs=1) as cci_pool,
            tc.tile_pool(name="xf", bufs=2) as xf_pool,
            tc.tile_pool(name="xb", bufs=2) as xb_pool,
            tc.tile_pool(name="xc", bufs=2) as xc_pool,
            tc.tile_pool(name="xt", bufs=1) as xt_pool,
            tc.tile_pool(name="u3", bufs=2) as u3_pool,
            tc.tile_pool(name="yo", bufs=4) as yo_pool,
            tc.tile_pool(name="psA", bufs=3, space="PSUM") as psA,
            tc.tile_pool(name="psB", bufs=3, space="PSUM") as psB,
        ):
            w1_sb = w1_pool.tile([128, k1, KW1], _BF16)
            nc.scalar.dma_start(w1_sb[:], w1_d.rearrange("(kt p) m -> p kt m", p=128))
            cci_sb = cci_pool.tile([128, 16], _I16)
            nc.scalar.dma_start(cci_sb[:], cci_d[:])

            u3_tiles = []
            for blk in range(n_blk):
                t0 = blk * BLK
                xt_sb = xt_pool.tile([128, k1, BLK], _BF16)
                u3_sb = u3_pool.tile([128, k2, BLK], _BF16)
                u3_tiles.append(u3_sb)
                xcbs = []
                for tt in range(tpb):
                    xf = xf_pool.tile([128, N], _F32)
                    nc.gpsimd.dma_start(
                        xf[:], x_d[t0 + tt * TT : t0 + (tt + 1) * TT, :]
                    )
                    xb = xb_pool.tile([128, N], _BF16)
                    nc.gpsimd.tensor_copy(out=xb[:], in_=xf[:])
                    nc.sync.dma_start_transpose(
                        xt_sb[:, :, tt * TT : (tt + 1) * TT], xb[:]
                    )
                    # comp-column gather: [128 tok, 256]; transpose emitted
                    # after the big ones (it only gates MM-B)
                    xc = xc_pool.tile([128, NCOMP], _F32, tag="xcf")
                    nc.gpsimd.ap_gather(
                        xc[:], xf[:], cci_sb[:],
                        channels=128, num_elems=N, d=1, num_idxs=NCOMP,
                    )
                    xcb = xc_pool.tile([128, NCOMP], _BF16, tag="xcb")
                    nc.gpsimd.tensor_copy(out=xcb[:], in_=xc[:])
                    xcbs.append(xcb)
                for tt, xcb in enumerate(xcbs):
                    nc.sync.dma_start_transpose(
                        u3_sb[:, 4:6, tt * TT : (tt + 1) * TT], xcb[:]
                    )

                halves = [(0, BLK)] if blk > 0 else [(0, BLK // 2), (BLK // 2, BLK // 2)]
                for h0, hw_ in halves:
                    for mi, m in enumerate(range(m1)):
                        ps = psA.tile([128, BLK], _F32, tag="psA")
                        for kt in range(k1):
                            nc.tensor.matmul(
                                ps[:, :hw_],
                                w1_sb[:, kt, m * 128 : (m + 1) * 128],
                                xt_sb[:, kt, h0 : h0 + hw_],
                                start=(kt == 0),
                                stop=(kt == k1 - 1),
                            )
                        nc.vector.tensor_copy(
                            out=u3_sb[:, m_slot[mi], h0 : h0 + hw_], in_=ps[:, :hw_]
                        )

            for n in range(n2):
                w2_sb = w2_pool.tile([128, k2, BLK], _BF16)
                nc.scalar.dma_start(
                    w2_sb[:],
                    w2_d.rearrange("(kt p) n -> p kt n", p=128)[
                        :, :, n * BLK : (n + 1) * BLK
                    ],
                )
                for blk in range(n_blk):
                    t0 = blk * BLK
                    u3_sb = u3_tiles[blk]
                    for mt in range(tpb):
                        ps = psB.tile([128, BLK], _F32)
                        for kt in range(k2):
                            nc.tensor.matmul(
                                ps[:],
                                u3_sb[:, kt, mt * TT : (mt + 1) * TT],
                                w2_sb[:, kt, :],
                                start=(kt == 0),
                                stop=(kt == k2 - 1),
                            )
                        yo = yo_pool.tile([128, BLK], _F32)
                        nc.vector.tensor_copy(out=yo[:], in_=ps[:])
                        nc.scalar.dma_start(
                            y_d[
                                t0 + mt * TT : t0 + (mt + 1) * TT,
                                n * BLK : (n + 1) * BLK,
                            ],
                            yo[:],
                        )
    nc.finalize()
    return nc


_NC_CACHE = {}


def get_nc():
    if "nc" not in _NC_CACHE:
        _NC_CACHE["nc"] = _build_nc()
    return _NC_CACHE["nc"]


def _prep_weights(A, B, sparse_weights1, sparse_weights2, weights_norms_rowwise,
                  col_idx, col_comp_idx, row_idx, row_comp_idx):
    bf16 = ml_dtypes.bfloat16
    # W1' = [Bs; sparse2]  (768, 4096)
    w1 = np.zeros((KW1, N), dtype=np.float32)
    w1[:RANK, col_idx] = B * weights_norms_rowwise[None, :]
    w1[RANK:, :] = sparse_weights2
    # W2.T (1024, 4096): u = [hidden | x_comp | y_comp]
    w2t = np.zeros((KU, N), dtype=np.float32)
    w2t[:RANK, row_idx] = A.T
    w2t[RANK : RANK + NCOMP, row_idx] = sparse_weights1.T
    w2t[RANK + NCOMP + np.arange(NCOMP), row_comp_idx] = 1.0
    w1t = np.ascontiguousarray(w1.T).astype(bf16)
    w2t = np.ascontiguousarray(w2t).astype(bf16)
    # ap_gather index tile: idx j = s*16+p read from cci[p, s]
    cci = np.asarray(col_comp_idx, np.int64).reshape(16, 16).T.astype(np.int16)
    cci = np.ascontiguousarray(np.broadcast_to(cci[None], (8, 16, 16)).reshape(128, 16))
    return w1t, w2t, cci


def kernel(x, A, B, sparse_weights1, sparse_weights2, weights_norms_rowwise,
           col_idx, col_comp_idx, row_idx, row_comp_idx):
    x = np.asarray(x, dtype=np.float32)
    w1t, w2t, cci = _prep_weights(
        np.asarray(A, np.float32), np.asarray(B, np.float32),
        np.asarray(sparse_weights1, np.float32),
        np.asarray(sparse_weights2, np.float32),
        np.asarray(weights_norms_rowwise, np.float32),
        np.asarray(col_idx), np.asarray(col_comp_idx),
        np.asarray(row_idx), np.asarray(row_comp_idx),
    )
    nc = get_nc()
    xs = np.ascontiguousarray(x.reshape(TOK, N))
    in_maps = [
        {"x": xs[c * TPC : (c + 1) * TPC], "w1t": w1t, "w2t": w2t, "cci": cci}
        for c in range(N_CORES)
    ]
    res = run_bass_kernel_spmd(nc, in_maps, core_ids=list(range(N_CORES)))
    globals()["_LAST_RESULTS"] = res
    y = np.concatenate([res.results[c]["y"] for c in range(N_CORES)], axis=0)
    return np.ascontiguousarray(y.reshape(x.shape).astype(np.float32))


# revision 9
# speedup vs baseline: 1.6074x; 1.0051x over previous
"""Low_Rank_linear Trainium2 kernel, v7.

Two dense bf16 matmuls with all gather/scatter folded into host-built
weights; x transposed on-chip via batched xbar DMA-transposes.

Per-core pipeline (1024 tokens, 2 blocks of 512):
  block 1: x loaded f32 (SWDGE), cast on DVE (idle at startup), xbar
           transpose -> xt; MM-A split into 2 half-token groups so the
           PE starts after 2 transposes.
  block 2: xb filled by casting SWDGE DMAs straight from DRAM (no engine
           time), transposed after block-1 MM-A frees xt.
  comp columns: gpsimd ap_gather from the f32 tiles, cast, small
           transpose straight into the u3 stationary buffer.
  MM-B: n-chunk outer, both blocks inner; W2 streamed once.
"""

import numpy as np
import ml_dtypes

import concourse.bacc as bacc
import concourse.tile as tile
import concourse.mybir as mybir
from concourse.bass_utils import run_bass_kernel_spmd

N_CORES = 8
TOK = 8192
TPC = TOK // N_CORES
N = 4096
RANK = 512
NCOMP = 256
KU = RANK + NCOMP + NCOMP   # 1024: u width (MM-B contraction)
KW1 = RANK + NCOMP          # 768: MM-A output features (Bs + sparse2)
BLK = 512
TT = 128

_BF16 = mybir.dt.bfloat16
_F32 = mybir.dt.float32
_I16 = mybir.dt.int16


def _build_nc():
    nc = bacc.Bacc(None)
    x_d = nc.dram_tensor("x", [TPC, N], _F32, kind="ExternalInput")
    w1_d = nc.dram_tensor("w1t", [N, KW1], _BF16, kind="ExternalInput")
    w2_d = nc.dram_tensor("w2t", [KU, N], _BF16, kind="ExternalInput")
    cci_d = nc.dram_tensor("cci", [128, 16], _I16, kind="ExternalInput")
    y_d = nc.dram_tensor("y", [TPC, N], _F32, kind="ExternalOutput")

    n_blk = TPC // BLK          # 2
    tpb = BLK // TT             # 4
    k1 = N // 128               # 32
    m1 = KW1 // 128             # 6
    k2 = KU // 128              # 8
    n2 = N // BLK               # 8
    m_slot = [0, 1, 2, 3, 6, 7]

    with tile.TileContext(nc) as tc:
        with (
            tc.tile_pool(name="w1", bufs=1) as w1_pool,
            tc.tile_pool(name="w2", bufs=2) as w2_pool,
            tc.tile_pool(name="cci", bufs=1) as cci_pool,
            tc.tile_pool(name="xf", bufs=2) as xf_pool,
            tc.tile_pool(name="xb", bufs=4) as xb_pool,
            tc.tile_pool(name="xc", bufs=2) as xc_pool,
            tc.tile_pool(name="xt", bufs=1) as xt_pool,
            tc.tile_pool(name="u3", bufs=2) as u3_pool,
            tc.tile_pool(name="yo", bufs=2) as yo_pool,
            tc.tile_pool(name="psA", bufs=3, space="PSUM") as psA,
            tc.tile_pool(name="psB", bufs=3, space="PSUM") as psB,
        ):
            w1_sb = w1_pool.tile([128, k1, KW1], _BF16)
            nc.scalar.dma_start(w1_sb[:], w1_d.rearrange("(kt p) m -> p kt m", p=128))
            cci_sb = cci_pool.tile([128, 16], _I16)
            nc.scalar.dma_start(cci_sb[:], cci_d[:])

            u3_tiles = []
            for blk in range(n_blk):
                t0 = blk * BLK
                xt_sb = xt_pool.tile([128, k1, BLK], _BF16)
                u3_sb = u3_pool.tile([128, k2, BLK], _BF16)
                u3_tiles.append(u3_sb)
                xcbs = []
                for tt in range(tpb):
                    xf = xf_pool.tile([128, N], _F32)
                    nc.gpsimd.dma_start(
                        xf[:], x_d[t0 + tt * TT : t0 + (tt + 1) * TT, :]
                    )
                    xb = xb_pool.tile([128, N], _BF16)
                    if blk == 0:
                        # startup block: cast on DVE (idle this early)
                        nc.vector.tensor_copy(out=xb[:], in_=xf[:])
                    else:
                        # steady state: casting SWDGE DMA, no engine time
                        nc.gpsimd.dma_start(
                            xb[:], x_d[t0 + tt * TT : t0 + (tt + 1) * TT, :]
                        )
                    nc.sync.dma_start_transpose(
                        xt_sb[:, :, tt * TT : (tt + 1) * TT], xb[:]
                    )
                    # comp columns: gather from f32 tile, cast, stash
                    xc = xc_pool.tile([128, NCOMP], _F32, tag="xcf")
                    nc.gpsimd.ap_gather(
                        xc[:], xf[:], cci_sb[:],
                        channels=128, num_elems=N, d=1, num_idxs=NCOMP,
                    )
                    xcb = xc_pool.tile([128, NCOMP], _BF16, tag="xcb")
                    nc.vector.tensor_copy(out=xcb[:], in_=xc[:])
                    xcbs.append(xcb)
                for tt, xcb in enumerate(xcbs):
                    nc.sync.dma_start_transpose(
                        u3_sb[:, 4:6, tt * TT : (tt + 1) * TT], xcb[:]
                    )

                halves = (
                    [(0, BLK)] if blk > 0 else [(0, BLK // 2), (BLK // 2, BLK // 2)]
                )
                for h0, hw_ in halves:
                    for mi, m in enumerate(range(m1)):
                        ps = psA.tile([128, BLK], _F32, tag="psA")
                        for kt in range(k1):
                            nc.tensor.matmul(
                                ps[:, :hw_],
                                w1_sb[:, kt, m * 128 : (m + 1) * 128],
                                xt_sb[:, kt, h0 : h0 + hw_],
                                start=(kt == 0),
                                stop=(kt == k1 - 1),
                            )
                        nc.vector.tensor_copy(
                            out=u3_sb[:, m_slot[mi], h0 : h0 + hw_], in_=ps[:, :hw_]
                        )

            for n in range(n2):
                w2_sb = w2_pool.tile([128, k2, BLK], _BF16)
                nc.scalar.dma_start(
                    w2_sb[:],
                    w2_d.rearrange("(kt p) n -> p kt n", p=128)[
                        :, :, n * BLK : (n + 1) * BLK
                    ],
                )
                for blk in range(n_blk):
                    t0 = blk * BLK
                    u3_sb = u3_tiles[blk]
                    for mt in range(tpb):
                        ps = psB.tile([128, BLK], _F32)
                        for kt in range(k2):
                            nc.tensor.matmul(
                                ps[:],
                                u3_sb[:, kt, mt * TT : (mt + 1) * TT],
                                w2_sb[:, kt, :],
                                start=(kt == 0),
                                stop=(kt == k2 - 1),
                            )
                        yo = yo_pool.tile([128, BLK], _F32)
                        nc.vector.tensor_copy(out=yo[:], in_=ps[:])
                        nc.scalar.dma_start(
                            y_d[
                                t0 + mt * TT : t0 + (mt + 1) * TT,
                                n * BLK : (n + 1) * BLK,
                            ],
                            yo[:],
                        )
    nc.finalize()
    return nc


_NC_CACHE = {}


def get_nc():
    if "nc" not in _NC_CACHE:
        _NC_CACHE["nc"] = _build_nc()
    return _NC_CACHE["nc"]


def _prep_weights(A, B, sparse_weights1, sparse_weights2, weights_norms_rowwise,
                  col_idx, col_comp_idx, row_idx, row_comp_idx):
    bf16 = ml_dtypes.bfloat16
    # W1' = [Bs; sparse2]  (768, 4096)
    w1 = np.zeros((KW1, N), dtype=np.float32)
    w1[:RANK, col_idx] = B * weights_norms_rowwise[None, :]
    w1[RANK:, :] = sparse_weights2
    # W2.T (1024, 4096): u = [hidden | x_comp | y_comp]
    w2t = np.zeros((KU, N), dtype=np.float32)
    w2t[:RANK, row_idx] = A.T
    w2t[RANK : RANK + NCOMP, row_idx] = sparse_weights1.T
    w2t[RANK + NCOMP + np.arange(NCOMP), row_comp_idx] = 1.0
    w1t = np.ascontiguousarray(w1.T).astype(bf16)
    w2t = np.ascontiguousarray(w2t).astype(bf16)
    # ap_gather index tile: idx j = s*16+p read from cci[p, s]
    cci = np.asarray(col_comp_idx, np.int64).reshape(16, 16).T.astype(np.int16)
    cci = np.ascontiguousarray(np.broadcast_to(cci[None], (8, 16, 16)).reshape(128, 16))
    return w1t, w2t, cci


def kernel(x, A, B, sparse_weights1, sparse_weights2, weights_norms_rowwise,
           col_idx, col_comp_idx, row_idx, row_comp_idx):
    x = np.asarray(x, dtype=np.float32)
    w1t, w2t, cci = _prep_weights(
        np.asarray(A, np.float32), np.asarray(B, np.float32),
        np.asarray(sparse_weights1, np.float32),
        np.asarray(sparse_weights2, np.float32),
        np.asarray(weights_norms_rowwise, np.float32),
        np.asarray(col_idx), np.asarray(col_comp_idx),
        np.asarray(row_idx), np.asarray(row_comp_idx),
    )
    nc = get_nc()
    xs = np.ascontiguousarray(x.reshape(TOK, N))
    in_maps = [
        {"x": xs[c * TPC : (c + 1) * TPC], "w1t": w1t, "w2t": w2t, "cci": cci}
        for c in range(N_CORES)
    ]
    res = run_bass_kernel_spmd(nc, in_maps, core_ids=list(range(N_CORES)))
    globals()["_LAST_RESULTS"] = res
    y = np.concatenate([res.results[c]["y"] for c in range(N_CORES)], axis=0)
    return np.ascontiguousarray(y.reshape(x.shape).astype(np.float32))
